# revision 33
# baseline (speedup 1.0000x reference)
"""8-core Trainium2 Bass kernel for nn_Attention_89489938579587.

reference: qkv = x @ w_attn.T; split q,k,v per 16 heads (HD=128); RoPE
(interleaved pairs); non-causal SDPA; y @ w_proj.T.  B=4, T=2048, D=2048.

Sharding: core i -> batch b=i//2, token half i%2 (1024 tokens).  Each core
computes Q,K,V only for its OWN 1024 tokens (no duplicated K/V work), RoPEs
q/k with its local positions, then pair-AllGathers K,V with its batch
partner (4 collectives, one per 4-head group, overlapped with the Q
projection).  SDPA runs over all 16 heads x local queries x full 2048 kv;
output projection produces the core's token rows.  Host concatenates
8 x [1024, 2048].

All matmuls run in bf16 (separate LDWEIGHTS is pulled ahead by the PE's
reorder window, unlike fp32r's self-loading matmuls; fp32 PSUM accumulate;
~6e-3 rel err).  Q stays SBUF-resident; K/V round-trip DRAM through the
collective.  Weights are host-prepped into partition-major bf16 slabs.

Self-contained: builds the Bass program on first call, runs via
run_bass_kernel_spmd on cores 0-7.
"""

import numpy as np
from contextlib import ExitStack

import concourse.bass as bass
import concourse.tile as tile
from concourse import mybir
from concourse.bass import ts

# ---------------------------------------------------------------------------
# Workarounds for this toolchain:
# 1) walrus here rejects any instruction with >1 semaphore wait ("Too many
#    sync wait commands").  After Tile lowering, split extra waits onto
#    same-engine InstNoOp instructions inserted right before the offender.
# 2) the Tile tail drain carries many waits; patch _drain_and_barrier to put
#    them on SP nops (one each) before a waitless drain.
# ---------------------------------------------------------------------------
import bass_rust


def _split_multi_waits(nc, max_waits=1):
    n = 0
    for fn in nc.m.functions:
        for blk in fn.blocks:
            insts = blk.instructions
            i = 0
            while i < len(insts):
                inst = insts[i]
                si = inst.sync_info
                waits = list(si.on_wait) if (si is not None and si.on_wait) else []
                if len(waits) > max_waits:
                    si.on_wait = waits[:max_waits]
                    extra = waits[max_waits:]
                    for j in range(0, len(extra), max_waits):
                        nop = mybir.InstNoOp(
                            name=nc.get_next_instruction_name(), ins=[], outs=[])
                        nop.engine = inst.engine
                        nop.sync_info = bass_rust.SyncInfo(
                            on_wait=extra[j:j + max_waits], on_update=[])
                        nc.register_instruction(nop, overwrite=True)
                        insts.insert(i, nop)
                        i += 1
                        n += 1
                i += 1
    return n


def _patched_drain_and_barrier(self, tick_clock, wait_clock):
    from concourse.vector_clock import ScopedClock
    nc = self.nc
    probe = nc.sync.nop()
    wait_clock.add_sem_waits(probe.ins, ScopedClock({None: tick_clock.global_clock}))
    si = probe.ins.sync_info
    waits = list(si.on_wait or []) if si is not None else []
    if len(waits) > 1:
        si.on_wait = [waits[0]]
        for w in waits[1:]:
            nop = nc.sync.nop()
            nsi = nop.ins.sync_info
            if nsi is None:
                nop.ins.sync_info = bass_rust.SyncInfo(on_wait=[w], on_update=[])
            else:
                nsi.on_wait = [w]
    nc.sync.drain()
    nc.all_engine_barrier()
    assert self.sems is not None
    popped = nc._tile_sem_poison_stack.pop()
    assert popped is self._sem_poison
    nc.clear_and_free_semaphores(list(self.sems.allocated().values()))
    nc.all_engine_barrier()


_patched = False


def _apply_patches():
    global _patched
    if not _patched:
        tile.TileContext._drain_and_barrier = _patched_drain_and_barrier
        _patched = True


# ---------------------------------------------------------------------------
# Problem constants (hardcoded per spec)
# ---------------------------------------------------------------------------
F32R = mybir.dt.float32r
F32 = mybir.dt.float32
BF16 = mybir.dt.bfloat16
EXP = mybir.ActivationFunctionType.Exp

B, T, D, H, HD = 4, 2048, 2048, 16, 128
CC = D // 128           # contraction chunks
NTQ = 1024              # tokens per core (queries AND local kv)
NTKV = T                # full kv tokens after the pair gather
KC = NTKV // 128        # kv chunks in SDPA
KCL = NTQ // 128        # local kv chunks
SCALE = 1.0 / float(np.sqrt(HD))
N_CORES = 8
GROUPS = [[0, 1], [2, 3], [4, 5], [6, 7]]


def build_nc(n_cores=N_CORES):
    _apply_patches()
    nc = bass.Bass("TRN2", target_bir_lowering=False, debug=False,
                   num_devices=n_cores)
    # x columns: this core's 1024 tokens (transposed, bf16)
    xT = nc.dram_tensor("xT", [D, NTQ], BF16, kind="ExternalInput").ap()
    # partition-major weight slabs (bf16)
    wqs = nc.dram_tensor("wqs", [H, 128, CC * 128], BF16, kind="ExternalInput").ap()
    wks = nc.dram_tensor("wks", [H, 128, CC * 128], BF16, kind="ExternalInput").ap()
    wvs = nc.dram_tensor("wvs", [4, 4, 128, 4 * 512], BF16, kind="ExternalInput").ap()
    wps = nc.dram_tensor("wps", [4, 4, 128, 4 * 512], BF16, kind="ExternalInput").ap()
    # RoPE tables for the local positions (rotate-half form)
    cs2 = nc.dram_tensor("cs2", [128, NTQ], F32, kind="ExternalInput").ap()
    sn2 = nc.dram_tensor("sn2", [128, NTQ], F32, kind="ExternalInput").ap()
    onesd = nc.dram_tensor("onesd", [128, 128], F32R, kind="ExternalInput").ap()
    eyed = nc.dram_tensor("eyed", [128, 128], BF16, kind="ExternalInput").ap()
    out = nc.dram_tensor("out", [NTQ, D], F32, kind="ExternalOutput").ap()

    # collective bounce buffers: [group][k/v][head-in-group][128][NTQ]
    kvloc = nc.dram_tensor("kvloc", [4, 2, 4, 128, NTQ], BF16).ap()
    kfull = nc.dram_tensor("kfull", [4, 2, 4, 128, NTQ], BF16).ap()
    vfull = nc.dram_tensor("vfull", [4, 2, 4, 128, NTQ], BF16).ap()

    xT_r = xT.rearrange("(cc p) t -> cc p t", p=128)

    with tile.TileContext(nc) as tc, ExitStack() as octx:
        per_pool = octx.enter_context(tc.tile_pool(name="per", bufs=1))
        cs_sb = per_pool.tile([128, NTQ], F32, tag="cs")
        sn_sb = per_pool.tile([128, NTQ], F32, tag="sn")
        qT_all = per_pool.tile([128, H, NTQ], BF16, tag="qT")
        oT_all = per_pool.tile([128, H, NTQ], BF16, tag="oT")
        ones128 = per_pool.tile([128, 1], F32R, tag="o128")
        ones1 = per_pool.tile([1, 128], F32R, tag="o1")
        eye_sb = per_pool.tile([128, 128], BF16, tag="eye")
        # small constants ride the vector engine's DMA queue so they do not
        # delay the x-tile / weight-slab loads on the sync queue
        nc.scalar.dma_start(cs_sb[:], cs2[:])
        nc.scalar.dma_start(sn_sb[:], sn2[:])
        nc.scalar.dma_start(ones128[:], onesd[:, 0:1])
        nc.scalar.dma_start(ones1[:], onesd[0:1, :])
        nc.scalar.dma_start(eye_sb[:], eyed[:])
        # SDPA K/V tiles live outside phase-1's SBUF so their loads can
        # start as soon as each pair-gather lands (no aliasing on x tiles)
        kh_pool = octx.enter_context(tc.tile_pool(name="kh", bufs=2))
        vg_pool = octx.enter_context(tc.tile_pool(name="vg", bufs=2))

        # ---------------- phase 1: QKV projections + RoPE + gather ---------
        with ExitStack() as p1:
            xt_pool = p1.enter_context(tc.tile_pool(name="xt", bufs=16))
            wqk_pool = p1.enter_context(tc.tile_pool(name="wqk", bufs=5))
            wv_pool = p1.enter_context(tc.tile_pool(name="wv", bufs=5))
            ev_pool = p1.enter_context(tc.tile_pool(name="ev", bufs=8))
            rp_pool = p1.enter_context(tc.tile_pool(name="rp", bufs=2))
            ps1 = p1.enter_context(tc.tile_pool(name="ps1", bufs=6, space="PSUM"))

            def rope_rotate(ps, toff):
                """psum [128,512] -> rotated-sum bf16 parts (ta+tb) in rp tiles."""
                sf = rp_pool.tile([128, 512], F32, tag="sf")
                nc.scalar.copy(sf[:], ps[:])
                sw = rp_pool.tile([128, 512], F32, tag="sw")
                nc.gpsimd.dma_start(sw[0:64, :], sf[64:128, :])
                nc.gpsimd.dma_start(sw[64:128, :], sf[0:64, :])
                ta = rp_pool.tile([128, 512], BF16, tag="ta")
                nc.vector.tensor_mul(ta[:], sf[:], cs_sb[:, toff:toff + 512])
                tb = rp_pool.tile([128, 512], BF16, tag="tb")
                nc.vector.tensor_mul(tb[:], sw[:], sn_sb[:, toff:toff + 512])
                return ta, tb

            # local x tiles, shared by the Q, K and V passes
            xq = []
            for cc in range(CC):
                t_ = xt_pool.tile([128, 1024], BF16, tag="xt")
                nc.sync.dma_start(t_[:], xT_r[cc])
                xq.append(t_)

            # --- K then V per 4-head group, AllGather per group ---
            for g in range(4):
                for hh in range(4):
                    h = 4 * g + hh
                    wsl = wqk_pool.tile([128, CC, 128], BF16, tag="wqk")
                    for q4 in range(4):
                        nc.sync.dma_start(wsl[:, 4 * q4:4 * q4 + 4, :],
                                          wks[h, :, q4 * 512:(q4 + 1) * 512])
                    for tt in range(2):
                        ps = ps1.tile([128, 512], F32, tag="ps1")
                        for cc in range(CC):
                            nc.tensor.matmul(ps[:], wsl[:, cc, :],
                                             xq[cc][:, ts(tt, 512)],
                                             start=(cc == 0), stop=(cc == CC - 1))
                        ta, tb = rope_rotate(ps, tt * 512)
                        o = ev_pool.tile([128, 512], BF16, tag="ev")
                        nc.vector.tensor_add(o[:], ta[:], tb[:])
                        nc.gpsimd.dma_start(
                            kvloc[g, 0, hh, :, tt * 512:(tt + 1) * 512], o[:])
                nc.gpsimd.collective_compute(
                    "AllGather", mybir.AluOpType.bypass, replica_groups=GROUPS,
                    ins=[kvloc[g, 0].opt()], outs=[kfull[g].opt()])
                # V for this group's 512 feature columns (ft == g)
                wvl = []
                for qt in range(4):
                    w_ = wv_pool.tile([128, 4, 512], BF16, tag="wv")
                    nc.sync.dma_start(w_[:], wvs[g, qt])
                    wvl.append(w_)
                for tch in range(KCL):
                    ps = ps1.tile([128, 512], F32, tag="ps1")
                    for cc in range(CC):
                        wv_ap = wvl[cc // 4][:, cc % 4, :]
                        nc.tensor.matmul(ps[:], xq[cc][:, ts(tch, 128)], wv_ap,
                                         start=(cc == 0), stop=(cc == CC - 1))
                    o = ev_pool.tile([128, 512], BF16, tag="ev")
                    nc.scalar.copy(o[:], ps[:])
                    # V stored chunk-major: chunk tch at [g,1,tch//2,:,512*(tch%2)]
                    nc.gpsimd.dma_start(
                        kvloc[g, 1, tch // 2, :,
                              (tch % 2) * 512:(tch % 2 + 1) * 512], o[:])
                nc.gpsimd.collective_compute(
                    "AllGather", mybir.AluOpType.bypass, replica_groups=GROUPS,
                    ins=[kvloc[g, 1].opt()], outs=[vfull[g].opt()])

            # --- Q projection (overlaps the collectives) ---
            for h in range(H):
                wsl = wqk_pool.tile([128, CC, 128], BF16, tag="wqk")
                for q4 in range(4):
                    nc.sync.dma_start(wsl[:, 4 * q4:4 * q4 + 4, :],
                                      wqs[h, :, q4 * 512:(q4 + 1) * 512])
                for tt in range(2):
                    ps = ps1.tile([128, 512], F32, tag="ps1")
                    for cc in range(CC):
                        nc.tensor.matmul(ps[:], wsl[:, cc, :],
                                         xq[cc][:, ts(tt, 512)],
                                         start=(cc == 0), stop=(cc == CC - 1))
                    ta, tb = rope_rotate(ps, tt * 512)
                    nc.vector.tensor_add(qT_all[:, h, ts(tt, 512)], ta[:], tb[:])

        # ----- phase 2: SDPA per head, epilogue software-pipelined -----
        # wp slabs for phase 3 prefetch during SDPA from their own SBUF range
        wp_pool = octx.enter_context(tc.tile_pool(name="wp", bufs=8))
        wpl0 = []
        for qt in range(4):
            w_ = wp_pool.tile([128, 4, 512], BF16, tag="wp")
            nc.scalar.dma_start(w_[:], wps[0, qt])
            wpl0.append(w_)
        with ExitStack() as p2:
            e_pool = p2.enter_context(tc.tile_pool(name="eT", bufs=5))
            es_pool = p2.enter_context(tc.tile_pool(name="es", bufs=5))
            oe_pool = p2.enter_context(tc.tile_pool(name="oe", bufs=3))
            rs_pool = p2.enter_context(tc.tile_pool(name="rs", bufs=3))
            s_ps_pool = p2.enter_context(
                tc.tile_pool(name="sps", bufs=2, space="PSUM"))
            o_ps_pool = p2.enter_context(
                tc.tile_pool(name="ops", bufs=2, space="PSUM"))
            m_ps_pool = p2.enter_context(
                tc.tile_pool(name="mps", bufs=2, space="PSUM"))

            pendingA = []
            pendingB = []
            tail_work = [None]

            recip_work = []

            def stage_a():
                # partition-reduce the denominator; the reciprocal is split
                # into [1,128] quarters drip-fed between the DVE folds so a
                # 3.3us recip blob never stalls the strict-FIFO DVE queue
                h, qh, es_ps, oev = pendingA.pop(0)
                es_sb = rs_pool.tile([128, 512], F32R, tag="esb")
                nc.scalar.copy(es_sb[:], es_ps[:])
                nc.tensor.matmul(es_ps[0:1, :], ones128[:], es_sb[:],
                                 start=True, stop=True)
                rs = rs_pool.tile([1, 512], F32R, tag="rs")
                for q in range(4):
                    def quarter(q=q, rs=rs, es_ps=es_ps):
                        with nc.allow_low_precision(reason="f32r is 4-byte"):
                            nc.vector.reciprocal(
                                rs[:, q * 128:(q + 1) * 128],
                                es_ps[0:1, q * 128:(q + 1) * 128])
                    recip_work.append(quarter)
                pendingB.append((h, qh, oev, es_ps, rs))

            def stage_b():
                # broadcast the (long-ready) reciprocal, scale into oT
                h, qh, oev, es_ps, rs = pendingB.pop(0)
                nc.tensor.matmul(es_ps[:, :], ones1[:], rs[:],
                                 start=True, stop=True)
                nc.vector.tensor_mul(oT_all[:, h, ts(qh, 512)], oev[:],
                                     es_ps[:, :])

            # V for a 4-head group stays SBUF-resident: [128, kc, 512]
            vg_sb = None
            for h in range(H):
                g, hh = h // 4, h % 4
                if hh == 0:
                    vg_sb = vg_pool.tile([128, KC, 512], BF16, tag="vg")
                    for half in range(2):
                        for tch in range(KCL):
                            nc.sync.dma_start(
                                vg_sb[:, half * KCL + tch, :],
                                vfull[g, half, tch // 2, :,
                                      (tch % 2) * 512:(tch % 2 + 1) * 512])
                kh_sb = kh_pool.tile([128, NTKV], BF16, tag="kh")
                nc.sync.dma_start(kh_sb[:, 0:NTQ], kfull[g, 0, hh])
                nc.sync.dma_start(kh_sb[:, NTQ:NTKV], kfull[g, 1, hh])
                for qh in range(NTQ // 512):
                    qsl = qT_all[:, h, ts(qh, 512)]
                    o_ps = o_ps_pool.tile([128, 512], F32, tag="ops")
                    es_ps = m_ps_pool.tile([128, 512], F32, tag="mps")
                    NP = KC // 2  # kc pairs; exp runs on [128, 1024]
                    eTs = [None] * NP
                    gs = [None] * NP
                    vsl = vg_sb

                    def pv_pair(j, last, vsl=vsl, o_ps=o_ps, eTs=eTs, hh=hh):
                        for u in range(2):
                            kc = 2 * j + u
                            nc.tensor.matmul(
                                o_ps[:], vsl[:, kc, hh * 128:(hh + 1) * 128],
                                eTs[j][:, ts(u, 512)],
                                start=(kc == 0), stop=(last and u == 1))

                    def id_mm(j, es_ps=es_ps, gs=gs):
                        # accumulate pair j's folded denominator via identity
                        nc.tensor.matmul(es_ps[:], eye_sb[:], gs[j][:],
                                         start=(j == 0), stop=(j == NP - 1))

                    for j in range(NP):
                        s_ps = s_ps_pool.tile([128, 1024], F32, tag="sps")
                        nc.tensor.matmul(s_ps[:, 0:512],
                                         kh_sb[:, ts(2 * j, 128)], qsl,
                                         start=True, stop=True)
                        nc.tensor.matmul(s_ps[:, 512:1024],
                                         kh_sb[:, ts(2 * j + 1, 128)], qsl,
                                         start=True, stop=True)
                        eT = e_pool.tile([128, 1024], BF16, tag="eT")
                        nc.scalar.activation(eT[:], s_ps[:], EXP, scale=SCALE)
                        eTs[j] = eT
                        # fold pair j's two halves (DVE), accumulate on PE
                        gj = es_pool.tile([128, 512], BF16, tag="es")
                        nc.vector.tensor_add(gj[:], eT[:, 0:512],
                                             eT[:, 512:1024])
                        gs[j] = gj
                        if j == 2 and tail_work[0] is not None:
                            # previous chunk's tail hides under our QK 0-2,
                            # covering the ACT backlog ahead of its last exp
                            tail_work[0]()
                            tail_work[0] = None
                        if j >= 2:
                            pv_pair(j - 2, last=False)
                            id_mm(j - 2)
                            if j == 2 and pendingA:
                                stage_a()  # previous chunk's reduce + recip
                            if recip_work:
                                recip_work.pop(0)()

                    def tail(h=h, qh=qh, es_ps=es_ps, o_ps=o_ps,
                             pv_pair=pv_pair, id_mm=id_mm):
                        pv_pair(NP - 2, last=False)
                        id_mm(NP - 2)
                        pv_pair(NP - 1, last=True)
                        id_mm(NP - 1)
                        oev = oe_pool.tile([128, 512], BF16, tag="oe")
                        nc.scalar.copy(oev[:], o_ps[:])
                        pendingA.append((h, qh, es_ps, oev))
                        if pendingB:
                            stage_b()  # earlier chunk's broadcast + scale
                    tail_work[0] = tail
            if tail_work[0] is not None:
                tail_work[0]()
                tail_work[0] = None
            while pendingA:
                stage_a()
            while recip_work:
                recip_work.pop(0)()
            while pendingB:
                stage_b()

        # ----- phase 3: output projection -----
        with ExitStack() as p3:
            outev_pool = p3.enter_context(tc.tile_pool(name="outev", bufs=4))
            ps3 = p3.enter_context(tc.tile_pool(name="ps3", bufs=4, space="PSUM"))

            for ft in range(4):
                if ft == 0:
                    wpl = wpl0
                else:
                    wpl = []
                    for qt in range(4):
                        w_ = wp_pool.tile([128, 4, 512], BF16, tag="wp")
                        nc.sync.dma_start(w_[:], wps[ft, qt])
                        wpl.append(w_)
                for tch in range(NTQ // 128):
                    ps = ps3.tile([128, 512], F32, tag="ps3")
                    for hc in range(H):
                        wp_ap = wpl[hc // 4][:, hc % 4, :]
                        nc.tensor.matmul(ps[:], oT_all[:, hc, ts(tch, 128)],
                                         wp_ap,
                                         start=(hc == 0), stop=(hc == H - 1))
                    oev = outev_pool.tile([128, 512], F32, tag="outev")
                    nc.scalar.copy(oev[:], ps[:])
                    nc.gpsimd.dma_start(
                        out[ts(tch, 128), ft * 512:(ft + 1) * 512], oev[:])

    _split_multi_waits(nc)
    return nc


# ---------------------------------------------------------------------------
# host-side prep / assembly
# ---------------------------------------------------------------------------

_ONES = np.ones((128, 128), dtype=np.float32)


def prep_inputs(x, w_attn, w_proj):
    import ml_dtypes
    global _EYE
    _EYE = np.eye(128, dtype=ml_dtypes.bfloat16)
    bf16 = ml_dtypes.bfloat16
    x = np.asarray(x, dtype=np.float32)
    w_attn = np.asarray(w_attn, dtype=np.float32)
    w_proj = np.asarray(w_proj, dtype=np.float32)

    perm = np.concatenate([np.arange(0, HD, 2), np.arange(1, HD, 2)])
    colperm = (np.arange(H)[:, None] * HD + perm[None, :]).ravel()

    wq, wk, wv = w_attn[0:D], w_attn[D:2 * D], w_attn[2 * D:3 * D]
    # partition-major slabs: [h, p, cc*128] with wT[c, f] = w.T
    wqs = np.ascontiguousarray(
        wq.T[:, colperm].reshape(CC, 128, H, 128)
        .transpose(2, 1, 0, 3).reshape(H, 128, CC * 128)).astype(bf16)
    wks = np.ascontiguousarray(
        wk.T[:, colperm].reshape(CC, 128, H, 128)
        .transpose(2, 1, 0, 3).reshape(H, 128, CC * 128)).astype(bf16)
    # [ft, qt, p, 4*512]
    wvs = np.ascontiguousarray(
        wv.T.reshape(4, 4, 128, 4, 512)
        .transpose(3, 0, 2, 1, 4).reshape(4, 4, 128, 4 * 512)).astype(bf16)
    wps = np.ascontiguousarray(
        w_proj.T.reshape(4, 4, 128, 4, 512)
        .transpose(3, 0, 2, 1, 4).reshape(4, 4, 128, 4 * 512)).astype(bf16)

    inv = 1.0 / (10000.0 ** (np.arange(0, HD, 2, dtype=np.float64) / HD))
    fr = np.outer(np.arange(T, dtype=np.float64), inv)
    cos = np.cos(fr).T
    sin = np.sin(fr).T
    cs2 = np.concatenate([cos, cos], 0).astype(np.float32)
    sn2 = np.concatenate([-sin, sin], 0).astype(np.float32)

    in_maps = []
    for i in range(N_CORES):
        b, half = i // 2, i % 2
        q0 = half * NTQ
        xT_b = np.ascontiguousarray(x[b].T[:, q0:q0 + NTQ]).astype(bf16)
        in_maps.append({
            "xT": xT_b,
            "wqs": wqs, "wks": wks, "wvs": wvs, "wps": wps,
            "cs2": np.ascontiguousarray(cs2[:, q0:q0 + NTQ]),
            "sn2": np.ascontiguousarray(sn2[:, q0:q0 + NTQ]),
            "onesd": _ONES,
            "eyed": _EYE,
        })
    return in_maps


def assemble(results):
    out = np.empty((B, T, D), dtype=np.float32)
    for i in range(N_CORES):
        b, half = i // 2, i % 2
        out[b, half * NTQ:(half + 1) * NTQ, :] = results[i]["out"]
    return out


_nc_cache = None


def _get_nc():
    global _nc_cache
    if _nc_cache is None:
        _nc_cache = build_nc()
    return _nc_cache


def kernel(x, w_attn, w_proj):
    from concourse.bass_utils import run_bass_kernel_spmd
    nc = _get_nc()
    in_maps = prep_inputs(x, w_attn, w_proj)
    res = run_bass_kernel_spmd(nc, in_maps, list(range(N_CORES)))
    return assemble(res.results)


def run_profiled(x, w_attn, w_proj, trace_cores=None):
    """Like kernel() but with NTFF profiling; returns BassKernelResults."""
    from concourse.bass_utils import run_bass_kernel_spmd
    import sys as _sys, types as _types
    try:
        import antenv
        if "antenv.axon_hooks" not in _sys.modules:
            mod = _types.ModuleType("antenv.axon_hooks")
            _h = [None]
            mod.set_axon_ntff_profile_hook = lambda h: _h.__setitem__(0, h)
            mod.get_axon_ntff_profile_hook = lambda: _h[0]
            _sys.modules["antenv.axon_hooks"] = mod
            antenv.axon_hooks = mod
            from trn_agent_boot.trn_boot import _ntff_profile_via_ctypes
            mod.set_axon_ntff_profile_hook(
                _ntff_profile_via_ctypes('/opt/axon/libaxon_pjrt.so'))
    except Exception as e:  # profiling is best-effort
        print("profile hook setup failed:", e)
    nc = _get_nc()
    in_maps = prep_inputs(x, w_attn, w_proj)
    return run_bass_kernel_spmd(
        nc, in_maps, list(range(N_CORES)), trace=True,
        trace_cores=trace_cores if trace_cores is not None else [0])


# revision 35
# speedup vs baseline: 1.1563x; 1.1563x over previous
"""8-core Trainium2 Bass kernel for nn_Attention_89489938579587.

reference: qkv = x @ w_attn.T; split q,k,v per 16 heads (HD=128); RoPE
(interleaved pairs); non-causal SDPA; y @ w_proj.T.  B=4, T=2048, D=2048.

Sharding: core i -> batch b=i//2, token half i%2 (1024 tokens).  Each core
computes Q,K,V only for its OWN 1024 tokens (no duplicated K/V work), RoPEs
q/k with its local positions, then pair-AllGathers K,V with its batch
partner (4 collectives, one per 4-head group, overlapped with the Q
projection).  SDPA runs over all 16 heads x local queries x full 2048 kv;
output projection produces the core's token rows.  Host concatenates
8 x [1024, 2048].

All matmuls run in bf16 (separate LDWEIGHTS is pulled ahead by the PE's
reorder window, unlike fp32r's self-loading matmuls; fp32 PSUM accumulate;
~6e-3 rel err).  Q stays SBUF-resident; K/V round-trip DRAM through the
collective.  Weights are host-prepped into partition-major bf16 slabs.

Self-contained: builds the Bass program on first call, runs via
run_bass_kernel_spmd on cores 0-7.
"""

import numpy as np
from contextlib import ExitStack

import concourse.bass as bass
import concourse.tile as tile
from concourse import mybir
from concourse.bass import ts

# ---------------------------------------------------------------------------
# Workarounds for this toolchain:
# 1) walrus here rejects any instruction with >1 semaphore wait ("Too many
#    sync wait commands").  After Tile lowering, split extra waits onto
#    same-engine InstNoOp instructions inserted right before the offender.
# 2) the Tile tail drain carries many waits; patch _drain_and_barrier to put
#    them on SP nops (one each) before a waitless drain.
# ---------------------------------------------------------------------------
import bass_rust


def _split_multi_waits(nc, max_waits=1):
    n = 0
    for fn in nc.m.functions:
        for blk in fn.blocks:
            insts = blk.instructions
            i = 0
            while i < len(insts):
                inst = insts[i]
                si = inst.sync_info
                waits = list(si.on_wait) if (si is not None and si.on_wait) else []
                if len(waits) > max_waits:
                    si.on_wait = waits[:max_waits]
                    extra = waits[max_waits:]
                    for j in range(0, len(extra), max_waits):
                        nop = mybir.InstNoOp(
                            name=nc.get_next_instruction_name(), ins=[], outs=[])
                        nop.engine = inst.engine
                        nop.sync_info = bass_rust.SyncInfo(
                            on_wait=extra[j:j + max_waits], on_update=[])
                        nc.register_instruction(nop, overwrite=True)
                        insts.insert(i, nop)
                        i += 1
                        n += 1
                i += 1
    return n


def _patched_drain_and_barrier(self, tick_clock, wait_clock):
    from concourse.vector_clock import ScopedClock
    nc = self.nc
    probe = nc.sync.nop()
    wait_clock.add_sem_waits(probe.ins, ScopedClock({None: tick_clock.global_clock}))
    si = probe.ins.sync_info
    waits = list(si.on_wait or []) if si is not None else []
    if len(waits) > 1:
        si.on_wait = [waits[0]]
        for w in waits[1:]:
            nop = nc.sync.nop()
            nsi = nop.ins.sync_info
            if nsi is None:
                nop.ins.sync_info = bass_rust.SyncInfo(on_wait=[w], on_update=[])
            else:
                nsi.on_wait = [w]
    nc.sync.drain()
    nc.all_engine_barrier()
    assert self.sems is not None
    popped = nc._tile_sem_poison_stack.pop()
    assert popped is self._sem_poison
    nc.clear_and_free_semaphores(list(self.sems.allocated().values()))
    nc.all_engine_barrier()


_patched = False


def _apply_patches():
    global _patched
    if not _patched:
        tile.TileContext._drain_and_barrier = _patched_drain_and_barrier
        _patched = True


# ---------------------------------------------------------------------------
# Problem constants (hardcoded per spec)
# ---------------------------------------------------------------------------
F32R = mybir.dt.float32r
F32 = mybir.dt.float32
BF16 = mybir.dt.bfloat16
EXP = mybir.ActivationFunctionType.Exp

B, T, D, H, HD = 4, 2048, 2048, 16, 128
CC = D // 128           # contraction chunks
NTQ = 1024              # tokens per core (queries AND local kv)
NTKV = T                # full kv tokens after the pair gather
KC = NTKV // 128        # kv chunks in SDPA
KCL = NTQ // 128        # local kv chunks
SCALE = 1.0 / float(np.sqrt(HD))
N_CORES = 8
GROUPS = [[0, 1], [2, 3], [4, 5], [6, 7]]


def build_nc(n_cores=N_CORES):
    _apply_patches()
    nc = bass.Bass("TRN2", target_bir_lowering=False, debug=False,
                   num_devices=n_cores)
    # x columns: this core's 1024 tokens (transposed, bf16)
    xT = nc.dram_tensor("xT", [D, NTQ], BF16, kind="ExternalInput").ap()
    # partition-major weight slabs (bf16)
    wqs = nc.dram_tensor("wqs", [H, 128, CC * 128], BF16, kind="ExternalInput").ap()
    wks = nc.dram_tensor("wks", [H, 128, CC * 128], BF16, kind="ExternalInput").ap()
    wvs = nc.dram_tensor("wvs", [4, 4, 128, 4 * 512], BF16, kind="ExternalInput").ap()
    wps = nc.dram_tensor("wps", [4, 4, 128, 4 * 512], BF16, kind="ExternalInput").ap()
    # RoPE tables for the local positions (rotate-half form)
    cs2 = nc.dram_tensor("cs2", [128, NTQ], F32, kind="ExternalInput").ap()
    sn2 = nc.dram_tensor("sn2", [128, NTQ], F32, kind="ExternalInput").ap()
    onesd = nc.dram_tensor("onesd", [128, 128], F32R, kind="ExternalInput").ap()
    eyed = nc.dram_tensor("eyed", [128, 128], BF16, kind="ExternalInput").ap()
    out = nc.dram_tensor("out", [NTQ, D], F32, kind="ExternalOutput").ap()

    # collective bounce buffers: [group][k/v][head-in-group][128][NTQ]
    kvloc = nc.dram_tensor("kvloc", [4, 2, 4, 128, NTQ], BF16).ap()
    kfull = nc.dram_tensor("kfull", [4, 2, 4, 128, NTQ], BF16).ap()
    vfull = nc.dram_tensor("vfull", [4, 2, 4, 128, NTQ], BF16).ap()

    xT_r = xT.rearrange("(cc p) t -> cc p t", p=128)

    with tile.TileContext(nc) as tc, ExitStack() as octx:
        per_pool = octx.enter_context(tc.tile_pool(name="per", bufs=1))
        cs_sb = per_pool.tile([128, NTQ], F32, tag="cs")
        sn_sb = per_pool.tile([128, NTQ], F32, tag="sn")
        qT_all = per_pool.tile([128, H, NTQ], BF16, tag="qT")
        oT_all = per_pool.tile([128, H, NTQ], BF16, tag="oT")
        ones128 = per_pool.tile([128, 1], F32R, tag="o128")
        ones1 = per_pool.tile([1, 128], F32R, tag="o1")
        eye_sb = per_pool.tile([128, 128], BF16, tag="eye")
        # small constants ride the vector engine's DMA queue so they do not
        # delay the x-tile / weight-slab loads on the sync queue
        nc.scalar.dma_start(cs_sb[:], cs2[:])
        nc.scalar.dma_start(sn_sb[:], sn2[:])
        nc.scalar.dma_start(ones128[:], onesd[:, 0:1])
        nc.scalar.dma_start(ones1[:], onesd[0:1, :])
        nc.scalar.dma_start(eye_sb[:], eyed[:])
        # SDPA K/V tiles live outside phase-1's SBUF so their loads can
        # start as soon as each pair-gather lands (no aliasing on x tiles)
        kh_pool = octx.enter_context(tc.tile_pool(name="kh", bufs=2))
        vg_pool = octx.enter_context(tc.tile_pool(name="vg", bufs=2))

        # ---------------- phase 1: QKV projections + RoPE + gather ---------
        with ExitStack() as p1:
            xt_pool = p1.enter_context(tc.tile_pool(name="xt", bufs=16))
            wqk_pool = p1.enter_context(tc.tile_pool(name="wqk", bufs=5))
            wv_pool = p1.enter_context(tc.tile_pool(name="wv", bufs=5))
            ev_pool = p1.enter_context(tc.tile_pool(name="ev", bufs=8))
            rp_pool = p1.enter_context(tc.tile_pool(name="rp", bufs=2))
            ps1 = p1.enter_context(tc.tile_pool(name="ps1", bufs=6, space="PSUM"))

            def rope_rotate(ps, toff):
                """psum [128,512] -> rotated-sum bf16 parts (ta+tb) in rp tiles."""
                sf = rp_pool.tile([128, 512], F32, tag="sf")
                nc.scalar.copy(sf[:], ps[:])
                sw = rp_pool.tile([128, 512], F32, tag="sw")
                nc.gpsimd.dma_start(sw[0:64, :], sf[64:128, :])
                nc.gpsimd.dma_start(sw[64:128, :], sf[0:64, :])
                ta = rp_pool.tile([128, 512], BF16, tag="ta")
                nc.vector.tensor_mul(ta[:], sf[:], cs_sb[:, toff:toff + 512])
                tb = rp_pool.tile([128, 512], BF16, tag="tb")
                nc.vector.tensor_mul(tb[:], sw[:], sn_sb[:, toff:toff + 512])
                return ta, tb

            # local x tiles, shared by the Q, K and V passes
            xq = []
            for cc in range(CC):
                t_ = xt_pool.tile([128, 1024], BF16, tag="xt")
                nc.sync.dma_start(t_[:], xT_r[cc])
                xq.append(t_)

            # --- K then V per 4-head group, AllGather per group ---
            for g in range(4):
                for hh in range(4):
                    h = 4 * g + hh
                    wsl = wqk_pool.tile([128, CC, 128], BF16, tag="wqk")
                    for q4 in range(4):
                        nc.sync.dma_start(wsl[:, 4 * q4:4 * q4 + 4, :],
                                          wks[h, :, q4 * 512:(q4 + 1) * 512])
                    for tt in range(2):
                        ps = ps1.tile([128, 512], F32, tag="ps1")
                        for cc in range(CC):
                            nc.tensor.matmul(ps[:], wsl[:, cc, :],
                                             xq[cc][:, ts(tt, 512)],
                                             start=(cc == 0), stop=(cc == CC - 1))
                        ta, tb = rope_rotate(ps, tt * 512)
                        o = ev_pool.tile([128, 512], BF16, tag="ev")
                        nc.vector.tensor_add(o[:], ta[:], tb[:])
                        nc.gpsimd.dma_start(
                            kvloc[g, 0, hh, :, tt * 512:(tt + 1) * 512], o[:])
                nc.gpsimd.collective_compute(
                    "AllGather", mybir.AluOpType.bypass, replica_groups=GROUPS,
                    ins=[kvloc[g, 0].opt()], outs=[kfull[g].opt()])
                # V for this group's 512 feature columns (ft == g)
                wvl = []
                for qt in range(4):
                    w_ = wv_pool.tile([128, 4, 512], BF16, tag="wv")
                    nc.sync.dma_start(w_[:], wvs[g, qt])
                    wvl.append(w_)
                for tch in range(KCL):
                    ps = ps1.tile([128, 512], F32, tag="ps1")
                    for cc in range(CC):
                        wv_ap = wvl[cc // 4][:, cc % 4, :]
                        nc.tensor.matmul(ps[:], xq[cc][:, ts(tch, 128)], wv_ap,
                                         start=(cc == 0), stop=(cc == CC - 1))
                    o = ev_pool.tile([128, 512], BF16, tag="ev")
                    nc.scalar.copy(o[:], ps[:])
                    # V stored chunk-major: chunk tch at [g,1,tch//2,:,512*(tch%2)]
                    nc.gpsimd.dma_start(
                        kvloc[g, 1, tch // 2, :,
                              (tch % 2) * 512:(tch % 2 + 1) * 512], o[:])
                nc.gpsimd.collective_compute(
                    "AllGather", mybir.AluOpType.bypass, replica_groups=GROUPS,
                    ins=[kvloc[g, 1].opt()], outs=[vfull[g].opt()])

            # --- Q projection (overlaps the collectives) ---
            for h in range(H):
                wsl = wqk_pool.tile([128, CC, 128], BF16, tag="wqk")
                for q4 in range(4):
                    nc.sync.dma_start(wsl[:, 4 * q4:4 * q4 + 4, :],
                                      wqs[h, :, q4 * 512:(q4 + 1) * 512])
                for tt in range(2):
                    ps = ps1.tile([128, 512], F32, tag="ps1")
                    for cc in range(CC):
                        nc.tensor.matmul(ps[:], wsl[:, cc, :],
                                         xq[cc][:, ts(tt, 512)],
                                         start=(cc == 0), stop=(cc == CC - 1))
                    ta, tb = rope_rotate(ps, tt * 512)
                    nc.vector.tensor_add(qT_all[:, h, ts(tt, 512)], ta[:], tb[:])

        # ----- phase 2: SDPA per head, epilogue software-pipelined -----
        # wp slabs for phase 3 prefetch during SDPA from their own SBUF range
        wp_pool = octx.enter_context(tc.tile_pool(name="wp", bufs=8))
        wpl0 = []
        for qt in range(4):
            w_ = wp_pool.tile([128, 4, 512], BF16, tag="wp")
            nc.scalar.dma_start(w_[:], wps[0, qt])
            wpl0.append(w_)
        with ExitStack() as p2:
            e_pool = p2.enter_context(tc.tile_pool(name="eT", bufs=5))
            es_pool = p2.enter_context(tc.tile_pool(name="es", bufs=5))
            oe_pool = p2.enter_context(tc.tile_pool(name="oe", bufs=3))
            rs_pool = p2.enter_context(tc.tile_pool(name="rs", bufs=3))
            s_ps_pool = p2.enter_context(
                tc.tile_pool(name="sps", bufs=2, space="PSUM"))
            o_ps_pool = p2.enter_context(
                tc.tile_pool(name="ops", bufs=2, space="PSUM"))
            m_ps_pool = p2.enter_context(
                tc.tile_pool(name="mps", bufs=2, space="PSUM"))

            pendingA = []
            pendingB = []
            tail_work = [None]

            recip_work = []

            def stage_a():
                # partition-reduce the denominator; the reciprocal is split
                # into [1,128] quarters drip-fed between the DVE folds so a
                # 3.3us recip blob never stalls the strict-FIFO DVE queue
                h, qh, es_ps, oev = pendingA.pop(0)
                es_sb = rs_pool.tile([128, 512], F32R, tag="esb")
                nc.scalar.copy(es_sb[:], es_ps[:])
                nc.tensor.matmul(es_ps[0:1, :], ones128[:], es_sb[:],
                                 start=True, stop=True)
                rs = rs_pool.tile([1, 512], F32R, tag="rs")
                for q in range(4):
                    def quarter(q=q, rs=rs, es_ps=es_ps):
                        with nc.allow_low_precision(reason="f32r is 4-byte"):
                            nc.vector.reciprocal(
                                rs[:, q * 128:(q + 1) * 128],
                                es_ps[0:1, q * 128:(q + 1) * 128])
                    recip_work.append(quarter)
                pendingB.append((h, qh, oev, es_ps, rs))

            def stage_b():
                # broadcast the (long-ready) reciprocal, scale into oT
                h, qh, oev, es_ps, rs = pendingB.pop(0)
                nc.tensor.matmul(es_ps[:, :], ones1[:], rs[:],
                                 start=True, stop=True)
                nc.vector.tensor_mul(oT_all[:, h, ts(qh, 512)], oev[:],
                                     es_ps[:, :])

            # V for a 4-head group stays SBUF-resident: [128, kc, 512]
            vg_sb = None
            for h in range(H):
                g, hh = h // 4, h % 4
                if hh == 0:
                    vg_sb = vg_pool.tile([128, KC, 512], BF16, tag="vg")
                    for half in range(2):
                        for tch in range(KCL):
                            nc.gpsimd.dma_start(
                                vg_sb[:, half * KCL + tch, :],
                                vfull[g, half, tch // 2, :,
                                      (tch % 2) * 512:(tch % 2 + 1) * 512])
                kh_sb = kh_pool.tile([128, NTKV], BF16, tag="kh")
                nc.gpsimd.dma_start(kh_sb[:, 0:NTQ], kfull[g, 0, hh])
                nc.gpsimd.dma_start(kh_sb[:, NTQ:NTKV], kfull[g, 1, hh])
                for qh in range(NTQ // 512):
                    qsl = qT_all[:, h, ts(qh, 512)]
                    o_ps = o_ps_pool.tile([128, 512], F32, tag="ops")
                    es_ps = m_ps_pool.tile([128, 512], F32, tag="mps")
                    NP = KC // 2  # kc pairs; exp runs on [128, 1024]
                    eTs = [None] * NP
                    gs = [None] * NP
                    vsl = vg_sb

                    def pv_pair(j, last, vsl=vsl, o_ps=o_ps, eTs=eTs, hh=hh):
                        for u in range(2):
                            kc = 2 * j + u
                            nc.tensor.matmul(
                                o_ps[:], vsl[:, kc, hh * 128:(hh + 1) * 128],
                                eTs[j][:, ts(u, 512)],
                                start=(kc == 0), stop=(last and u == 1))

                    def id_mm(j, es_ps=es_ps, gs=gs):
                        # accumulate pair j's folded denominator via identity
                        nc.tensor.matmul(es_ps[:], eye_sb[:], gs[j][:],
                                         start=(j == 0), stop=(j == NP - 1))

                    for j in range(NP):
                        s_ps = s_ps_pool.tile([128, 1024], F32, tag="sps")
                        nc.tensor.matmul(s_ps[:, 0:512],
                                         kh_sb[:, ts(2 * j, 128)], qsl,
                                         start=True, stop=True)
                        nc.tensor.matmul(s_ps[:, 512:1024],
                                         kh_sb[:, ts(2 * j + 1, 128)], qsl,
                                         start=True, stop=True)
                        eT = e_pool.tile([128, 1024], BF16, tag="eT")
                        nc.scalar.activation(eT[:], s_ps[:], EXP, scale=SCALE)
                        eTs[j] = eT
                        # fold pair j's two halves (DVE), accumulate on PE
                        gj = es_pool.tile([128, 512], BF16, tag="es")
                        nc.vector.tensor_add(gj[:], eT[:, 0:512],
                                             eT[:, 512:1024])
                        gs[j] = gj
                        if j == 2 and tail_work[0] is not None:
                            # previous chunk's tail hides under our QK 0-2,
                            # covering the ACT backlog ahead of its last exp
                            tail_work[0]()
                            tail_work[0] = None
                        if j >= 2:
                            pv_pair(j - 2, last=False)
                            id_mm(j - 2)
                            if j == 2 and pendingA:
                                stage_a()  # previous chunk's reduce + recip
                            if recip_work:
                                recip_work.pop(0)()

                    def tail(h=h, qh=qh, es_ps=es_ps, o_ps=o_ps,
                             pv_pair=pv_pair, id_mm=id_mm):
                        pv_pair(NP - 2, last=False)
                        id_mm(NP - 2)
                        pv_pair(NP - 1, last=True)
                        id_mm(NP - 1)
                        oev = oe_pool.tile([128, 512], BF16, tag="oe")
                        nc.scalar.copy(oev[:], o_ps[:])
                        pendingA.append((h, qh, es_ps, oev))
                        if pendingB:
                            stage_b()  # earlier chunk's broadcast + scale
                    tail_work[0] = tail
            if tail_work[0] is not None:
                tail_work[0]()
                tail_work[0] = None
            while pendingA:
                stage_a()
            while recip_work:
                recip_work.pop(0)()
            while pendingB:
                stage_b()

        # ----- phase 3: output projection -----
        with ExitStack() as p3:
            outev_pool = p3.enter_context(tc.tile_pool(name="outev", bufs=4))
            ps3 = p3.enter_context(tc.tile_pool(name="ps3", bufs=4, space="PSUM"))

            for ft in range(4):
                if ft == 0:
                    wpl = wpl0
                else:
                    wpl = []
                    for qt in range(4):
                        w_ = wp_pool.tile([128, 4, 512], BF16, tag="wp")
                        nc.sync.dma_start(w_[:], wps[ft, qt])
                        wpl.append(w_)
                for tch in range(NTQ // 128):
                    ps = ps3.tile([128, 512], F32, tag="ps3")
                    for hc in range(H):
                        wp_ap = wpl[hc // 4][:, hc % 4, :]
                        nc.tensor.matmul(ps[:], oT_all[:, hc, ts(tch, 128)],
                                         wp_ap,
                                         start=(hc == 0), stop=(hc == H - 1))
                    oev = outev_pool.tile([128, 512], F32, tag="outev")
                    nc.scalar.copy(oev[:], ps[:])
                    nc.gpsimd.dma_start(
                        out[ts(tch, 128), ft * 512:(ft + 1) * 512], oev[:])

    _split_multi_waits(nc)
    return nc


# ---------------------------------------------------------------------------
# host-side prep / assembly
# ---------------------------------------------------------------------------

_ONES = np.ones((128, 128), dtype=np.float32)


def prep_inputs(x, w_attn, w_proj):
    import ml_dtypes
    global _EYE
    _EYE = np.eye(128, dtype=ml_dtypes.bfloat16)
    bf16 = ml_dtypes.bfloat16
    x = np.asarray(x, dtype=np.float32)
    w_attn = np.asarray(w_attn, dtype=np.float32)
    w_proj = np.asarray(w_proj, dtype=np.float32)

    perm = np.concatenate([np.arange(0, HD, 2), np.arange(1, HD, 2)])
    colperm = (np.arange(H)[:, None] * HD + perm[None, :]).ravel()

    wq, wk, wv = w_attn[0:D], w_attn[D:2 * D], w_attn[2 * D:3 * D]
    # partition-major slabs: [h, p, cc*128] with wT[c, f] = w.T
    wqs = np.ascontiguousarray(
        wq.T[:, colperm].reshape(CC, 128, H, 128)
        .transpose(2, 1, 0, 3).reshape(H, 128, CC * 128)).astype(bf16)
    wks = np.ascontiguousarray(
        wk.T[:, colperm].reshape(CC, 128, H, 128)
        .transpose(2, 1, 0, 3).reshape(H, 128, CC * 128)).astype(bf16)
    # [ft, qt, p, 4*512]
    wvs = np.ascontiguousarray(
        wv.T.reshape(4, 4, 128, 4, 512)
        .transpose(3, 0, 2, 1, 4).reshape(4, 4, 128, 4 * 512)).astype(bf16)
    wps = np.ascontiguousarray(
        w_proj.T.reshape(4, 4, 128, 4, 512)
        .transpose(3, 0, 2, 1, 4).reshape(4, 4, 128, 4 * 512)).astype(bf16)

    inv = 1.0 / (10000.0 ** (np.arange(0, HD, 2, dtype=np.float64) / HD))
    fr = np.outer(np.arange(T, dtype=np.float64), inv)
    cos = np.cos(fr).T
    sin = np.sin(fr).T
    cs2 = np.concatenate([cos, cos], 0).astype(np.float32)
    sn2 = np.concatenate([-sin, sin], 0).astype(np.float32)

    in_maps = []
    for i in range(N_CORES):
        b, half = i // 2, i % 2
        q0 = half * NTQ
        xT_b = np.ascontiguousarray(x[b].T[:, q0:q0 + NTQ]).astype(bf16)
        in_maps.append({
            "xT": xT_b,
            "wqs": wqs, "wks": wks, "wvs": wvs, "wps": wps,
            "cs2": np.ascontiguousarray(cs2[:, q0:q0 + NTQ]),
            "sn2": np.ascontiguousarray(sn2[:, q0:q0 + NTQ]),
            "onesd": _ONES,
            "eyed": _EYE,
        })
    return in_maps


def assemble(results):
    out = np.empty((B, T, D), dtype=np.float32)
    for i in range(N_CORES):
        b, half = i // 2, i % 2
        out[b, half * NTQ:(half + 1) * NTQ, :] = results[i]["out"]
    return out


_nc_cache = None


def _get_nc():
    global _nc_cache
    if _nc_cache is None:
        _nc_cache = build_nc()
    return _nc_cache


def kernel(x, w_attn, w_proj):
    from concourse.bass_utils import run_bass_kernel_spmd
    nc = _get_nc()
    in_maps = prep_inputs(x, w_attn, w_proj)
    res = run_bass_kernel_spmd(nc, in_maps, list(range(N_CORES)))
    return assemble(res.results)


def run_profiled(x, w_attn, w_proj, trace_cores=None):
    """Like kernel() but with NTFF profiling; returns BassKernelResults."""
    from concourse.bass_utils import run_bass_kernel_spmd
    import sys as _sys, types as _types
    try:
        import antenv
        if "antenv.axon_hooks" not in _sys.modules:
            mod = _types.ModuleType("antenv.axon_hooks")
            _h = [None]
            mod.set_axon_ntff_profile_hook = lambda h: _h.__setitem__(0, h)
            mod.get_axon_ntff_profile_hook = lambda: _h[0]
            _sys.modules["antenv.axon_hooks"] = mod
            antenv.axon_hooks = mod
            from trn_agent_boot.trn_boot import _ntff_profile_via_ctypes
            mod.set_axon_ntff_profile_hook(
                _ntff_profile_via_ctypes('/opt/axon/libaxon_pjrt.so'))
    except Exception as e:  # profiling is best-effort
        print("profile hook setup failed:", e)
    nc = _get_nc()
    in_maps = prep_inputs(x, w_attn, w_proj)
    return run_bass_kernel_spmd(
        nc, in_maps, list(range(N_CORES)), trace=True,
        trace_cores=trace_cores if trace_cores is not None else [0])
